# revision 4
# baseline (speedup 1.0000x reference)
"""AttentionBlock kernel for 8 Trainium2 NeuronCores.

Problem: B=16, C=256, H=W=32 (S=1024), 4 heads, d_k=64, f32.
  xs = x.reshape(B,C,S).T -> qkv proj -> per-head softmax(q k^T / 8) v
  -> out proj + residual -> [B, C, H, W]

Strategy: pure data-parallel over batch (2 batches per core, no
collectives). Per batch, everything is computed in transposed layouts so
no on-chip transposes are needed:
  - qT/kT [head-pair 128, S] from weight-stationary projection
  - v [S, (h d)] from x-stationary projection, augmented with a ones
    column per head so the attention matmul also produces the softmax
    denominator (row 64 of the psum accumulator)
  - scoresT[j, i] per head, exp on ScalarE with fused 1/sqrt(dk) scale
  - oT = v_aug^T @ expT ([65, S]: rows 0-63 = unnormalized o^T, row 64 =
    sumexp); normalize with reciprocal + gpsimd partition_broadcast
  - resT[c, i] = w_out^T @ oT + b_out + xT, DMA'd straight out
Matmuls run in float32r (hw fast-fp32 mode, 4x the fp32 throughput,
~1.5e-4 rel err).
"""

import sys

for _p in ("/opt/trn_rl_repo",):
    if _p not in sys.path:
        sys.path.insert(0, _p)

import numpy as np

import concourse.bass as bass
from concourse import bacc
import concourse.mybir as mybir
from concourse.tile import TileContext
from concourse.bass_utils import run_bass_kernel_spmd

F32 = mybir.dt.float32
F32R = mybir.dt.float32r

N_CORES = 8
B, C, H, W = 16, 256, 32, 32
S = H * W            # 1024
NH, DK = 4, 64       # heads, head dim
BPC = B // N_CORES   # batches per core
SCALE = DK ** -0.5

TRACE = False
LAST_RESULTS = None
_CACHED_NC = None


def build_nc():
    nc = bacc.Bacc()
    x_d = nc.declare_dram_parameter("x", [BPC, C, S], F32R, isOutput=False)
    wq_d = nc.declare_dram_parameter("wq", [C, 2 * DK * 2], F32R, isOutput=False)
    wk_d = nc.declare_dram_parameter("wk", [C, 2 * DK * 2], F32R, isOutput=False)
    wv_d = nc.declare_dram_parameter("wv", [C, NH * DK], F32R, isOutput=False)
    wo_d = nc.declare_dram_parameter("wo", [NH * DK, C], F32R, isOutput=False)
    bq_d = nc.declare_dram_parameter("bq", [NH * DK, 1], F32, isOutput=False)
    bk_d = nc.declare_dram_parameter("bk", [NH * DK, 1], F32, isOutput=False)
    bv_d = nc.declare_dram_parameter("bv", [NH * DK], F32, isOutput=False)
    bo_d = nc.declare_dram_parameter("bo", [C, 1], F32, isOutput=False)
    out_d = nc.declare_dram_parameter("out", [BPC, C, S], F32, isOutput=True)

    Exp = mybir.ActivationFunctionType.Exp
    ADD = mybir.AluOpType.add

    with TileContext(nc) as tc:
        with tc.tile_pool(name="consts", bufs=1) as consts, \
             tc.tile_pool(name="xp", bufs=4) as xp, \
             tc.tile_pool(name="qk", bufs=3) as qkp, \
             tc.tile_pool(name="vp", bufs=10) as vp, \
             tc.tile_pool(name="ex", bufs=10) as exp_pool, \
             tc.tile_pool(name="ot", bufs=3) as otp, \
             tc.tile_pool(name="sm", bufs=3) as smp, \
             tc.tile_pool(name="res", bufs=3) as resp, \
             tc.tile_pool(name="ps", bufs=2, space="PSUM") as psp:

            # ---- constants -------------------------------------------------
            wq_sb = []
            wk_sb = []
            wv_sb = []
            wo_sb = []
            bq_sb = []
            bk_sb = []
            bo_sb = []
            for c in range(2):
                t = consts.tile([128, 256], F32R, tag=f"wq{c}")
                nc.sync.dma_start(out=t, in_=wq_d[c * 128:(c + 1) * 128, :])
                wq_sb.append(t)
                t = consts.tile([128, 256], F32R, tag=f"wk{c}")
                nc.sync.dma_start(out=t, in_=wk_d[c * 128:(c + 1) * 128, :])
                wk_sb.append(t)
                t = consts.tile([128, 256], F32R, tag=f"wv{c}")
                nc.sync.dma_start(out=t, in_=wv_d[c * 128:(c + 1) * 128, :])
                wv_sb.append(t)
                t = consts.tile([128, 256], F32R, tag=f"wo{c}")
                nc.sync.dma_start(out=t, in_=wo_d[c * 128:(c + 1) * 128, :])
                wo_sb.append(t)
                t = consts.tile([128, 1], F32, tag=f"bq{c}")
                nc.sync.dma_start(out=t, in_=bq_d[c * 128:(c + 1) * 128, :])
                bq_sb.append(t)
                t = consts.tile([128, 1], F32, tag=f"bk{c}")
                nc.sync.dma_start(out=t, in_=bk_d[c * 128:(c + 1) * 128, :])
                bk_sb.append(t)
                t = consts.tile([128, 1], F32, tag=f"bo{c}")
                nc.sync.dma_start(out=t, in_=bo_d[c * 128:(c + 1) * 128, :])
                bo_sb.append(t)
            # v bias broadcast across partitions: [128, (h d)]
            bv_bc = consts.tile([128, NH, DK], F32, tag="bvbc")
            _bv_ap = bv_d[:]
            bv_src = bass.AP(
                tensor=_bv_ap.tensor,
                offset=_bv_ap.offset,
                ap=[[0, 128], [DK, NH], [1, DK]],
            )
            nc.sync.dma_start(out=bv_bc, in_=bv_src)

            for b in range(BPC):
                # ---- load x (also serves as xT for the residual) -----------
                x_sb = []
                for c in range(2):
                    t = xp.tile([128, S], F32R, tag="x")
                    nc.sync.dma_start(out=t, in_=x_d[b, c * 128:(c + 1) * 128, :])
                    x_sb.append(t)

                # ---- q/k projections: qT/kT [pair 128, S] ------------------
                qT_sb = []
                kT_sb = []
                for which, w_sb, b_sb, dst in ((0, wq_sb, bq_sb, qT_sb),
                                               (1, wk_sb, bk_sb, kT_sb)):
                    for m in range(2):
                        ps = psp.tile([128, S], F32, tag="psA")
                        for ih in range(2):
                            sl = slice(ih * 512, (ih + 1) * 512)
                            for c in range(2):
                                nc.tensor.matmul(
                                    ps[:, sl],
                                    w_sb[c][:, m * 128:(m + 1) * 128],
                                    x_sb[c][:, sl],
                                    start=(c == 0), stop=(c == 1),
                                )
                        t = qkp.tile([128, S], F32R, tag=f"qk{which}")
                        nc.vector.tensor_scalar_add(t, ps, b_sb[m])
                        dst.append(t)

                # ---- v projection: v_aug [s 128, h, 65] --------------------
                v_aug = []
                for s in range(8):
                    ps = psp.tile([128, NH * DK], F32, tag="psB")
                    for c in range(2):
                        nc.tensor.matmul(
                            ps,
                            x_sb[c][:, s * 128:(s + 1) * 128],
                            wv_sb[c],
                            start=(c == 0), stop=(c == 1),
                        )
                    t = vp.tile([128, NH, DK + 1], F32R, tag="vaug")
                    nc.vector.scalar_tensor_tensor(
                        out=t[:, :, 0:DK],
                        in0=ps.rearrange("p (h d) -> p h d", h=NH),
                        scalar=0.0,
                        in1=bv_bc,
                        op0=ADD, op1=ADD,
                    )
                    nc.vector.memset(t[:, :, DK:DK + 1].bitcast(F32), 1.0)
                    v_aug.append(t)

                # ---- attention per head ------------------------------------
                oT_sb = [None, None]
                for h in range(NH):
                    m, base = h // 2, 64 * (h % 2)
                    expT = []
                    for j in range(8):
                        ps = psp.tile([128, S], F32, tag="psA")
                        for ih in range(2):
                            sl = slice(ih * 512, (ih + 1) * 512)
                            nc.tensor.matmul(
                                ps[:, sl],
                                kT_sb[m][base:base + 64, j * 128:(j + 1) * 128],
                                qT_sb[m][base:base + 64, sl],
                                start=True, stop=True,
                            )
                        e = exp_pool.tile([128, S], F32R, tag="expT")
                        nc.scalar.activation(out=e, in_=ps, func=Exp, scale=SCALE)
                        expT.append(e)

                    ops = psp.tile([DK + 1, S], F32, tag="psB")
                    for j in range(8):
                        for ih in range(2):
                            sl = slice(ih * 512, (ih + 1) * 512)
                            nc.tensor.matmul(
                                ops[:, sl],
                                v_aug[j][:, h, :],
                                expT[j][:, sl],
                                start=(j == 0), stop=(j == 7),
                            )
                    rc = smp.tile([1, S], F32, tag="rc")
                    nc.vector.reciprocal(out=rc, in_=ops[DK:DK + 1, :])
                    rcb = smp.tile([64, S], F32, tag="rcb")
                    nc.gpsimd.partition_broadcast(rcb, rc)
                    if oT_sb[m] is None:
                        oT_sb[m] = otp.tile([128, S], F32R, tag="oT",
                                            name=f"oT_{b}_{m}")
                    nc.vector.tensor_mul(
                        out=oT_sb[m][base:base + 64, :],
                        in0=ops[0:DK, :],
                        in1=rcb,
                    )

                # ---- output projection + bias + residual -------------------
                for cc in range(2):
                    ps = psp.tile([128, S], F32, tag="psB")
                    for ih in range(2):
                        sl = slice(ih * 512, (ih + 1) * 512)
                        for k in range(2):
                            nc.tensor.matmul(
                                ps[:, sl],
                                wo_sb[k][:, cc * 128:(cc + 1) * 128],
                                oT_sb[k][:, sl],
                                start=(k == 0), stop=(k == 1),
                            )
                    r = resp.tile([128, S], F32, tag="res")
                    nc.vector.scalar_tensor_tensor(
                        out=r,
                        in0=ps,
                        scalar=bo_sb[cc],
                        in1=x_sb[cc].bitcast(F32),
                        op0=ADD, op1=ADD,
                    )
                    nc.sync.dma_start(
                        out=out_d[b, cc * 128:(cc + 1) * 128, :], in_=r,
                    )

    nc.finalize()
    return nc


def kernel(x, w_proj, b_proj, w_out, b_out):
    global LAST_RESULTS, _CACHED_NC
    x = np.ascontiguousarray(np.asarray(x, dtype=np.float32).reshape(B, C, S))
    w_proj = np.asarray(w_proj, dtype=np.float32)
    b_proj = np.asarray(b_proj, dtype=np.float32)
    w_out = np.ascontiguousarray(np.asarray(w_out, dtype=np.float32))
    b_out = np.asarray(b_out, dtype=np.float32)

    wp = w_proj.reshape(C, NH, 3, DK)
    wq = np.ascontiguousarray(wp[:, :, 0, :].reshape(C, NH * DK))
    wk = np.ascontiguousarray(wp[:, :, 1, :].reshape(C, NH * DK))
    wv = np.ascontiguousarray(wp[:, :, 2, :].reshape(C, NH * DK))
    bp = b_proj.reshape(NH, 3, DK)
    bq = np.ascontiguousarray(bp[:, 0, :].reshape(NH * DK, 1))
    bk = np.ascontiguousarray(bp[:, 1, :].reshape(NH * DK, 1))
    bv = np.ascontiguousarray(bp[:, 2, :].reshape(NH * DK))
    bo = np.ascontiguousarray(b_out.reshape(C, 1))

    if _CACHED_NC is None:
        _CACHED_NC = build_nc()
    nc = _CACHED_NC

    in_maps = []
    for i in range(N_CORES):
        in_maps.append({
            "x": x[i * BPC:(i + 1) * BPC],
            "wq": wq, "wk": wk, "wv": wv, "wo": w_out,
            "bq": bq, "bk": bk, "bv": bv, "bo": bo,
        })

    res = run_bass_kernel_spmd(nc, in_maps, core_ids=list(range(N_CORES)),
                               trace=TRACE)
    LAST_RESULTS = res
    out = np.concatenate([res.results[i]["out"] for i in range(N_CORES)], axis=0)
    return np.ascontiguousarray(out.reshape(B, C, H, W))
